# revision 15
# baseline (speedup 1.0000x reference)
"""DeepSeek-V3 MoE gate (sigmoid + group-restricted top-k routing) on 8 TRN2
NeuronCores.

Strategy (data-parallel over tokens, per sharding hint):
  - x [16384, 7168] f32 is sharded 2048 tokens/core; weight [256, 7168] and
    bias [256] are replicated.
  - Host pre-staging: x and w are transposed to [D, tokens]/[D, experts] and
    hi/lo fp16-split on the host (xh = f16(x*2^8), xl = f16(x*2^8 - xh);
    wh = f16(w*2^12), wl = f16(w*2^12 - wh)). Total staged bytes equal the
    fp32 originals (2 fp16 halves = 4 bytes), so HBM traffic is unchanged,
    but the device needs no transposes and no cast work.
  - Per core: logits*2^20 = sum_d [xh.wh + xh.wl] (hi PSUM bank) + [xl.wh]
    (lo PSUM bank); the dropped xl.wl term is ~2^-24 relative. Pure fp16
    matmuls at 1 cyc/row keep the PE at its 3-pass floor (~18.3us per
    128-token tile). Sigmoid (ACT LUT with the 2^-20 descale), then
    group-limited top-8 selection + weight normalization on the DVE via
    max8/max_index/match_replace.
  - Outputs (w [16384,8] f32, idx [16384,8] i32) are gathered host-side.
"""
import numpy as np

import concourse.bass as bass
import concourse.mybir as mybir
import concourse.tile as tile
from concourse import bacc
from concourse.bass_utils import run_bass_kernel_spmd

F32 = mybir.dt.float32
F16 = mybir.dt.float16
U32 = mybir.dt.uint32
I32 = mybir.dt.int32
AF = mybir.ActivationFunctionType
ALU = mybir.AluOpType
AX = mybir.AxisListType

N_CORES = 8
T = 16384
D = 7168
E = 256
TOPK = 8
N_GROUPS = 8
GSIZE = E // N_GROUPS       # 32
ROUTE_SCALE = 2.5

TPC = T // N_CORES          # 2048 tokens per core
ND = D // 128               # 56 contraction tiles
NT = TPC // 128             # 16 token tiles per core
WCH = 4                     # weight load chunks
NDC = ND // WCH             # 14 d-tiles per weight chunk

XSCALE = 2.0 ** 8           # x pre-scale (keeps fp16 lo-parts normal)
WSCALE = 2.0 ** 12          # w pre-scale
DESCALE = 1.0 / (XSCALE * WSCALE)

NEG_MASK = -1.0e30          # "-inf" for group masking
MARK = -3.0                 # match_replace marker (outside score range)

_CACHE = {}


def _routing_thunks(nc, rt, scores, biasr, OW, OI, i):
    """Group-limited top-8 routing for one tile of 128 tokens, returned as a
    list of emission thunks (kept as thunks so emission order on the DVE can
    be controlled by the caller)."""
    st = {}

    def t_s():
        st["s"] = rt.tile([128, E], F32, tag="s", name="s", bufs=1)
        nc.vector.tensor_add(st["s"][:], scores[:], biasr[:])

    def t_gmax1():
        st["gmax1"] = rt.tile([128, 8], F32, tag="gmax1", name="gmax1")
        nc.vector.tensor_reduce(
            st["gmax1"][:], st["s"].rearrange("p (g k) -> p g k", k=GSIZE),
            axis=AX.X, op=ALU.max,
        )

    def t_scr():
        st["scr"] = rt.tile([128, E], F32, tag="scr", name="scr", bufs=1)
        nc.vector.match_replace(
            st["scr"][:], in_to_replace=st["gmax1"][:], in_values=st["s"][:],
            imm_value=MARK,
        )

    def t_gsum():
        gmax2 = rt.tile([128, 8], F32, tag="gmax2", name="gmax2")
        nc.vector.tensor_reduce(
            gmax2[:], st["scr"].rearrange("p (g k) -> p g k", k=GSIZE),
            axis=AX.X, op=ALU.max,
        )
        st["gsum"] = rt.tile([128, 8], F32, tag="gsum", name="gsum")
        nc.vector.tensor_add(st["gsum"][:], st["gmax1"][:], gmax2[:])

    def t_pen():
        g8 = rt.tile([128, 8], F32, tag="g8", name="g8")
        nc.vector.max(g8[:], st["gsum"][:])
        st["pen"] = rt.tile([128, 8], F32, tag="pen", name="pen")
        nc.vector.tensor_scalar(
            st["pen"][:], st["gsum"][:], g8[:, 3:4], scalar2=NEG_MASK,
            op0=ALU.is_lt, op1=ALU.mult,
        )

    def t_masked():
        st["masked"] = rt.tile([128, E], F32, tag="masked", name="masked", bufs=1)
        pen3 = st["pen"].rearrange("p (g k) -> p g k", k=1).to_broadcast(
            [128, N_GROUPS, GSIZE]
        )
        nc.vector.tensor_tensor(
            st["masked"].rearrange("p (g k) -> p g k", k=GSIZE),
            st["s"].rearrange("p (g k) -> p g k", k=GSIZE), pen3, op=ALU.add,
        )

    def t_sel8():
        st["sel8"] = rt.tile([128, 8], F32, tag="sel8", name="sel8")
        nc.vector.max(st["sel8"][:], st["masked"][:])

    def t_idx8():
        st["idx8"] = rt.tile([128, 8], U32, tag="idx8", name="idx8")
        nc.vector.max_index(st["idx8"][:], st["sel8"][:], st["masked"][:])
        iout = rt.tile([128, TOPK], I32, tag="iout", name="iout")
        nc.vector.tensor_copy(iout[:], st["idx8"][:])
        nc.sync.dma_start(OI[bass.ts(i, 128), :], iout[:])

    def t_scr2():
        st["scr2"] = rt.tile([128, E], F32, tag="scr2", name="scr2", bufs=1)
        nc.vector.match_replace(
            st["scr2"][:], in_to_replace=st["sel8"][:], in_values=st["masked"][:],
            imm_value=MARK,
        )

    def t_mark():
        st["mark"] = rt.tile([128, E], F32, tag="mark", name="mark", bufs=1)
        nc.vector.tensor_scalar(
            st["mark"][:], st["scr2"][:], MARK, scalar2=None, op0=ALU.is_equal
        )

    def t_dsc():
        st["dsc"] = rt.tile([128, E], F32, tag="dsc", name="dsc", bufs=1)
        nc.vector.tensor_tensor(st["dsc"][:], scores[:], st["mark"][:], op=ALU.mult)

    def t_ssel8():
        st["ssel8"] = rt.tile([128, 8], F32, tag="ssel8", name="ssel8")
        nc.vector.max(st["ssel8"][:], st["dsc"][:])

    def t_isel8():
        st["isel8"] = rt.tile([128, 8], U32, tag="isel8", name="isel8")
        nc.vector.max_index(st["isel8"][:], st["ssel8"][:], st["dsc"][:])

    def t_casts():
        st["idx8f"] = rt.tile([128, 8], F32, tag="idx8f", name="idx8f")
        nc.vector.tensor_copy(st["idx8f"][:], st["idx8"][:])
        st["isel8f"] = rt.tile([128, 8], F32, tag="isel8f", name="isel8f")
        nc.vector.tensor_copy(st["isel8f"][:], st["isel8"][:])

    def t_eq():
        st["eq"] = rt.tile([128, 8, 8], F32, tag="eq", name="eq", bufs=1)
        idx8_b = st["idx8f"].rearrange("p (j k) -> p j k", k=1).to_broadcast(
            [128, 8, 8]
        )
        isel8_b = st["isel8f"].rearrange("p (k j) -> p k j", k=1).to_broadcast(
            [128, 8, 8]
        )
        nc.vector.tensor_tensor(st["eq"][:], idx8_b, isel8_b, op=ALU.is_equal)

    def t_wj():
        prod = rt.tile([128, 8, 8], F32, tag="prod", name="prod", bufs=1)
        ssel8_b = st["ssel8"].rearrange("p (k j) -> p k j", k=1).to_broadcast(
            [128, 8, 8]
        )
        nc.vector.tensor_tensor(prod[:], st["eq"][:], ssel8_b, op=ALU.mult)
        st["wj"] = rt.tile([128, 8], F32, tag="wj", name="wj")
        nc.vector.tensor_reduce(st["wj"][:], prod[:], axis=AX.X, op=ALU.add)

    def t_rec():
        sumw = rt.tile([128, 1], F32, tag="sumw", name="sumw")
        nc.vector.tensor_reduce(sumw[:], st["wj"][:], axis=AX.X, op=ALU.add)
        st["rec"] = rt.tile([128, 1], F32, tag="rec", name="rec")
        nc.vector.reciprocal(st["rec"][:], sumw[:])

    def t_out():
        wout = rt.tile([128, TOPK], F32, tag="wout", name="wout")
        nc.vector.tensor_scalar(
            wout[:], st["wj"][:], st["rec"][:, 0:1], scalar2=ROUTE_SCALE,
            op0=ALU.mult, op1=ALU.mult,
        )
        nc.sync.dma_start(OW[bass.ts(i, 128), :], wout[:])

    return [t_s, t_gmax1, t_scr, t_gsum, t_pen, t_masked, t_sel8, t_idx8,
            t_scr2, t_mark, t_dsc, t_ssel8, t_isel8, t_casts, t_eq, t_wj,
            t_rec, t_out]


def _build(tpc: int = TPC):
    """x and w arrive pre-transposed and fp16 hi/lo-split from the host:
      XHL [D, NT*256] f16: row d, tile i holds [xh(d, tok0:128) | xl(d, ...)]
      WHL [D, 512]    f16: row d holds [wh(d, e0:256) | wl(d, e0:256)]
    Per token tile the device does only: one DMA (512B descriptors), 168
    fp16 matmuls (56 d-tiles x {hi.wh, hi.wl} into the hi PSUM bank + 56
    {lo.wh} into the lo bank), hi+lo combine, sigmoid, DVE routing chain.
    """
    nt = tpc // 128
    npair = nt // 2
    nc = bacc.Bacc("TRN2", target_bir_lowering=False, debug=False)

    XH = nc.dram_tensor("XH", [D, tpc], F16, kind="ExternalInput")
    XL = nc.dram_tensor("XL", [D, tpc], F16, kind="ExternalInput")
    WH = nc.dram_tensor("WH", [D, E], F16, kind="ExternalInput")
    WL = nc.dram_tensor("WL", [D, E], F16, kind="ExternalInput")
    BIASR = nc.dram_tensor("BIASR", [128, E], F32, kind="ExternalInput")
    OW = nc.dram_tensor("OW", [tpc, TOPK], F32, kind="ExternalOutput")
    OI = nc.dram_tensor("OI", [tpc, TOPK], I32, kind="ExternalOutput")

    # DRAM views with the d-tile index split out: [128 p, n d-tiles, cols]
    xhv = XH.ap().rearrange("(n p) c -> p n c", p=128)
    xlv = XL.ap().rearrange("(n p) c -> p n c", p=128)
    whv = WH.ap().rearrange("(n p) c -> p n c", p=128)
    wlv = WL.ap().rearrange("(n p) c -> p n c", p=128)

    with tile.TileContext(nc) as tc:
        with (
            tc.tile_pool(name="consts", bufs=1) as consts,
            tc.tile_pool(name="wp", bufs=1) as wp,
            tc.tile_pool(name="xhp", bufs=3) as xhp,
            tc.tile_pool(name="xlp", bufs=2) as xlp,
            tc.tile_pool(name="rt", bufs=2) as rt,
            tc.tile_pool(name="psh", bufs=4, space="PSUM") as psh,
            tc.tile_pool(name="psl", bufs=2, space="PSUM") as psl,
            tc.tile_pool(name="psw", bufs=1, space="PSUM") as psw,
        ):
            biasr = consts.tile([128, E], F32)
            nc.sync.dma_start(biasr[:], BIASR[:])

            # PE p-state warm-up: ~11us of throwaway fp32 matmuls on biasr so
            # the HAM clock gate opens before the first real matmul arrives
            # (cold-start otherwise costs ~12us of 2-4x slower matmuls).
            scratch = psw.tile([128, E], F32)
            for _ in range(24):
                nc.tensor.matmul(scratch[:], biasr[:, 0:128], biasr[:, 0:E],
                                 start=True, stop=True)
            for _ in range(6):
                nc.tensor.matmul(scratch[:, 0:64], biasr[:, 0:128],
                                 biasr[:, 0:64], start=True, stop=True)

            wh = wp.tile([128, ND, E], F16)
            wl = wp.tile([128, ND, E], F16)
            H = ND // 2

            def load_w(half, lo):
                dst, src = (wl, wlv) if lo else (wh, whv)
                nc.sync.dma_start(
                    dst[:, half * H : (half + 1) * H, :],
                    src[:, half * H : (half + 1) * H, :],
                )

            xh_t = {}
            xl_t = {}

            def load_xh(j, half=None):
                if j not in xh_t:
                    xh_t[j] = xhp.tile([128, ND, 256], F16, tag="xh", name="xh")
                d0, d1 = (0, ND) if half is None else (half * H, (half + 1) * H)
                nc.sync.dma_start(
                    xh_t[j][:, d0:d1, :], xhv[:, d0:d1, bass.ts(j, 256)]
                )

            def load_xl(j):
                xl_t[j] = xlp.tile([128, ND, 256], F16, tag="xl", name="xl")
                nc.sync.dma_start(xl_t[j][:], xlv[:, :, bass.ts(j, 256)])

            hi_ps = {}
            lo_ps = {}

            def emit_hi(i, d0, d1):
                # per d-tile: xh.wh then xh.wl, same bank + accumulation
                # order as the proven baseline
                if i not in hi_ps:
                    hi_ps[i] = psh.tile([128, E], F32, tag="logits", name="logits")
                logits = hi_ps[i]
                xt = xh_t[i // 2]
                c = (i % 2) * 128
                for d in range(d0, d1):
                    nc.tensor.matmul(
                        logits[:], xt[:, d, c : c + 128], wh[:, d, :],
                        start=(d == 0), stop=False,
                    )
                    nc.tensor.matmul(
                        logits[:], xt[:, d, c : c + 128], wl[:, d, :],
                        start=False, stop=(d == ND - 1),
                    )

            def emit_lo(i):
                if i not in lo_ps:
                    lo_ps[i] = psl.tile([128, E], F32, tag="logits_lo",
                                        name="logits_lo")
                logits_lo = lo_ps[i]
                xt = xl_t[i // 2]
                c = (i % 2) * 128
                for d in range(ND):
                    nc.tensor.matmul(
                        logits_lo[:], xt[:, d, c : c + 128], wh[:, d, :],
                        start=(d == 0), stop=(d == ND - 1),
                    )

            def emit_cr(i):
                lo_sb = rt.tile([128, E], F32, tag="lo_sb", name="lo_sb", bufs=1)
                nc.scalar.copy(lo_sb[:], lo_ps.pop(i)[:])
                logsum = rt.tile([128, E], F32, tag="logsum", name="logsum", bufs=1)
                nc.vector.tensor_tensor(
                    logsum[:], hi_ps.pop(i)[:], lo_sb[:], op=ALU.add
                )
                scores = rt.tile([128, E], F32, tag="scores", name="scores", bufs=1)
                nc.scalar.activation(scores[:], logsum[:], AF.Sigmoid,
                                     scale=DESCALE)
                for t in _routing_thunks(nc, rt, scores, biasr, OW, OI, i):
                    t()

            # ---- startup DMA: hi-path first (wh + xh unlock 2/3 of the PE
            # work per byte), wl/xl trail behind
            load_w(0, lo=False)
            load_xh(0, half=0)
            load_w(1, lo=False)
            load_xh(0, half=1)
            load_xh(1)
            load_w(0, lo=True)
            load_w(1, lo=True)
            load_xl(0)
            # pair 0 d-half-interleaved so the PE runs continuously from the
            # first WH/XH chunk arrival
            emit_hi(0, 0, H)
            emit_hi(1, 0, H)
            emit_hi(0, H, ND)
            emit_hi(1, H, ND)
            emit_hi(2, 0, ND)
            emit_hi(3, 0, ND)
            # ---- steady state: iter j handles lo+routing of pair j and the
            # hi pass of pair j+2 (lag-2), prefetching one pair ahead
            for j in range(npair):
                if j + 2 < npair:
                    load_xh(j + 2)
                if j + 1 < npair:
                    load_xl(j + 1)
                emit_lo(2 * j)
                emit_cr(2 * j)
                emit_lo(2 * j + 1)
                emit_cr(2 * j + 1)
                if j + 2 < npair:
                    emit_hi(2 * (j + 2), 0, ND)
                    emit_hi(2 * (j + 2) + 1, 0, ND)

    nc.compile()
    return nc


def kernel(x: np.ndarray, weight: np.ndarray, bias: np.ndarray):
    x = np.ascontiguousarray(x, dtype=np.float32)
    weight = np.ascontiguousarray(weight, dtype=np.float32)
    bias = np.ascontiguousarray(bias, dtype=np.float32)

    if "nc" not in _CACHE:
        _CACHE["nc"] = _build()
    nc = _CACHE["nc"]

    # host staging: transpose + fp16 hi/lo split (same RNE rounding the
    # device ACT/DVE casts produced in the previous revision)
    xs = x * np.float32(XSCALE)
    xh = xs.astype(np.float16)
    xl = (xs - xh.astype(np.float32)).astype(np.float16)
    ws = weight * np.float32(WSCALE)
    wh = ws.astype(np.float16)
    wl = (ws - wh.astype(np.float32)).astype(np.float16)

    wh_t = np.ascontiguousarray(wh.T)
    wl_t = np.ascontiguousarray(wl.T)
    biasr = np.tile(bias[None, :], (128, 1))

    in_maps = []
    for c in range(N_CORES):
        sl = slice(c * TPC, (c + 1) * TPC)
        in_maps.append(
            {
                "XH": np.ascontiguousarray(xh[sl].T),
                "XL": np.ascontiguousarray(xl[sl].T),
                "WH": wh_t,
                "WL": wl_t,
                "BIASR": biasr,
            }
        )
    global _last_in_maps
    _last_in_maps = in_maps
    res = run_bass_kernel_spmd(nc, in_maps, core_ids=list(range(N_CORES)))
    w = np.concatenate([r["OW"] for r in res.results], axis=0)
    idx = np.concatenate([r["OI"] for r in res.results], axis=0)
    return w, idx


_last_in_maps = None


# revision 16
# speedup vs baseline: 1.0710x; 1.0710x over previous
"""DeepSeek-V3 MoE gate (sigmoid + group-restricted top-k routing) on 8 TRN2
NeuronCores.

Strategy (data-parallel over tokens, per sharding hint):
  - x [16384, 7168] f32 is sharded 2048 tokens/core; weight [256, 7168] and
    bias [256] are replicated.
  - Host pre-staging: x and w are transposed to [D, tokens]/[D, experts] and
    hi/lo fp16-split on the host (xh = f16(x*2^8), xl = f16(x*2^8 - xh);
    wh = f16(w*2^12), wl = f16(w*2^12 - wh)). Total staged bytes equal the
    fp32 originals (2 fp16 halves = 4 bytes), so HBM traffic is unchanged,
    but the device needs no transposes and no cast work.
  - Per core: logits*2^20 = sum_d [xh.wh + xh.wl] (hi PSUM bank) + [xl.wh]
    (lo PSUM bank); the dropped xl.wl term is ~2^-24 relative. Pure fp16
    matmuls at 1 cyc/row keep the PE at its 3-pass floor (~18.3us per
    128-token tile). Sigmoid (ACT LUT with the 2^-20 descale), then
    group-limited top-8 selection + weight normalization on the DVE via
    max8/max_index/match_replace.
  - Outputs (w [16384,8] f32, idx [16384,8] i32) are gathered host-side.
"""
import numpy as np

import concourse.bass as bass
import concourse.mybir as mybir
import concourse.tile as tile
from concourse import bacc
from concourse.bass_utils import run_bass_kernel_spmd

F32 = mybir.dt.float32
F16 = mybir.dt.float16
U32 = mybir.dt.uint32
I32 = mybir.dt.int32
AF = mybir.ActivationFunctionType
ALU = mybir.AluOpType
AX = mybir.AxisListType

N_CORES = 8
T = 16384
D = 7168
E = 256
TOPK = 8
N_GROUPS = 8
GSIZE = E // N_GROUPS       # 32
ROUTE_SCALE = 2.5

TPC = T // N_CORES          # 2048 tokens per core
ND = D // 128               # 56 contraction tiles
NT = TPC // 128             # 16 token tiles per core
WCH = 4                     # weight load chunks
NDC = ND // WCH             # 14 d-tiles per weight chunk

XSCALE = 2.0 ** 8           # x pre-scale (keeps fp16 lo-parts normal)
WSCALE = 2.0 ** 12          # w pre-scale
DESCALE = 1.0 / (XSCALE * WSCALE)

NEG_MASK = -1.0e30          # "-inf" for group masking
MARK = -3.0                 # match_replace marker (outside score range)

_CACHE = {}


def _routing_thunks(nc, rt, scores, biasr, OW, OI, i):
    """Group-limited top-8 routing for one tile of 128 tokens, returned as a
    list of emission thunks (kept as thunks so emission order on the DVE can
    be controlled by the caller)."""
    st = {}

    def t_s():
        st["s"] = rt.tile([128, E], F32, tag="s", name="s", bufs=1)
        nc.vector.tensor_add(st["s"][:], scores[:], biasr[:])

    def t_gmax1():
        st["gmax1"] = rt.tile([128, 8], F32, tag="gmax1", name="gmax1")
        nc.vector.tensor_reduce(
            st["gmax1"][:], st["s"].rearrange("p (g k) -> p g k", k=GSIZE),
            axis=AX.X, op=ALU.max,
        )

    def t_scr():
        st["scr"] = rt.tile([128, E], F32, tag="scr", name="scr", bufs=1)
        nc.vector.match_replace(
            st["scr"][:], in_to_replace=st["gmax1"][:], in_values=st["s"][:],
            imm_value=MARK,
        )

    def t_gsum():
        gmax2 = rt.tile([128, 8], F32, tag="gmax2", name="gmax2")
        nc.vector.tensor_reduce(
            gmax2[:], st["scr"].rearrange("p (g k) -> p g k", k=GSIZE),
            axis=AX.X, op=ALU.max,
        )
        st["gsum"] = rt.tile([128, 8], F32, tag="gsum", name="gsum")
        nc.vector.tensor_add(st["gsum"][:], st["gmax1"][:], gmax2[:])

    def t_pen():
        g8 = rt.tile([128, 8], F32, tag="g8", name="g8")
        nc.vector.max(g8[:], st["gsum"][:])
        st["pen"] = rt.tile([128, 8], F32, tag="pen", name="pen")
        nc.vector.tensor_scalar(
            st["pen"][:], st["gsum"][:], g8[:, 3:4], scalar2=NEG_MASK,
            op0=ALU.is_lt, op1=ALU.mult,
        )

    def t_masked():
        st["masked"] = rt.tile([128, E], F32, tag="masked", name="masked", bufs=1)
        pen3 = st["pen"].rearrange("p (g k) -> p g k", k=1).to_broadcast(
            [128, N_GROUPS, GSIZE]
        )
        nc.vector.tensor_tensor(
            st["masked"].rearrange("p (g k) -> p g k", k=GSIZE),
            st["s"].rearrange("p (g k) -> p g k", k=GSIZE), pen3, op=ALU.add,
        )

    def t_sel8():
        st["sel8"] = rt.tile([128, 8], F32, tag="sel8", name="sel8")
        nc.vector.max(st["sel8"][:], st["masked"][:])

    def t_idx8():
        st["idx8"] = rt.tile([128, 8], U32, tag="idx8", name="idx8")
        nc.vector.max_index(st["idx8"][:], st["sel8"][:], st["masked"][:])
        iout = rt.tile([128, TOPK], I32, tag="iout", name="iout")
        nc.vector.tensor_copy(iout[:], st["idx8"][:])
        nc.sync.dma_start(OI[bass.ts(i, 128), :], iout[:])

    def t_scr2():
        st["scr2"] = rt.tile([128, E], F32, tag="scr2", name="scr2", bufs=1)
        nc.vector.match_replace(
            st["scr2"][:], in_to_replace=st["sel8"][:], in_values=st["masked"][:],
            imm_value=MARK,
        )

    def t_mark():
        st["mark"] = rt.tile([128, E], F32, tag="mark", name="mark", bufs=1)
        nc.vector.tensor_scalar(
            st["mark"][:], st["scr2"][:], MARK, scalar2=None, op0=ALU.is_equal
        )

    def t_dsc():
        st["dsc"] = rt.tile([128, E], F32, tag="dsc", name="dsc", bufs=1)
        nc.vector.tensor_tensor(st["dsc"][:], scores[:], st["mark"][:], op=ALU.mult)

    def t_ssel8():
        st["ssel8"] = rt.tile([128, 8], F32, tag="ssel8", name="ssel8")
        nc.vector.max(st["ssel8"][:], st["dsc"][:])

    def t_isel8():
        st["isel8"] = rt.tile([128, 8], U32, tag="isel8", name="isel8")
        nc.vector.max_index(st["isel8"][:], st["ssel8"][:], st["dsc"][:])

    def t_casts():
        st["idx8f"] = rt.tile([128, 8], F32, tag="idx8f", name="idx8f")
        nc.vector.tensor_copy(st["idx8f"][:], st["idx8"][:])
        st["isel8f"] = rt.tile([128, 8], F32, tag="isel8f", name="isel8f")
        nc.vector.tensor_copy(st["isel8f"][:], st["isel8"][:])

    def t_eq():
        st["eq"] = rt.tile([128, 8, 8], F32, tag="eq", name="eq", bufs=1)
        idx8_b = st["idx8f"].rearrange("p (j k) -> p j k", k=1).to_broadcast(
            [128, 8, 8]
        )
        isel8_b = st["isel8f"].rearrange("p (k j) -> p k j", k=1).to_broadcast(
            [128, 8, 8]
        )
        nc.vector.tensor_tensor(st["eq"][:], idx8_b, isel8_b, op=ALU.is_equal)

    def t_wj():
        prod = rt.tile([128, 8, 8], F32, tag="prod", name="prod", bufs=1)
        ssel8_b = st["ssel8"].rearrange("p (k j) -> p k j", k=1).to_broadcast(
            [128, 8, 8]
        )
        nc.vector.tensor_tensor(prod[:], st["eq"][:], ssel8_b, op=ALU.mult)
        st["wj"] = rt.tile([128, 8], F32, tag="wj", name="wj")
        nc.vector.tensor_reduce(st["wj"][:], prod[:], axis=AX.X, op=ALU.add)

    def t_rec():
        sumw = rt.tile([128, 1], F32, tag="sumw", name="sumw")
        nc.vector.tensor_reduce(sumw[:], st["wj"][:], axis=AX.X, op=ALU.add)
        st["rec"] = rt.tile([128, 1], F32, tag="rec", name="rec")
        nc.vector.reciprocal(st["rec"][:], sumw[:])

    def t_out():
        wout = rt.tile([128, TOPK], F32, tag="wout", name="wout")
        nc.vector.tensor_scalar(
            wout[:], st["wj"][:], st["rec"][:, 0:1], scalar2=ROUTE_SCALE,
            op0=ALU.mult, op1=ALU.mult,
        )
        nc.sync.dma_start(OW[bass.ts(i, 128), :], wout[:])

    return [t_s, t_gmax1, t_scr, t_gsum, t_pen, t_masked, t_sel8, t_idx8,
            t_scr2, t_mark, t_dsc, t_ssel8, t_isel8, t_casts, t_eq, t_wj,
            t_rec, t_out]


def _build(tpc: int = TPC):
    """x and w arrive pre-transposed and fp16 hi/lo-split from the host:
      XHL [D, NT*256] f16: row d, tile i holds [xh(d, tok0:128) | xl(d, ...)]
      WHL [D, 512]    f16: row d holds [wh(d, e0:256) | wl(d, e0:256)]
    Per token tile the device does only: one DMA (512B descriptors), 168
    fp16 matmuls (56 d-tiles x {hi.wh, hi.wl} into the hi PSUM bank + 56
    {lo.wh} into the lo bank), hi+lo combine, sigmoid, DVE routing chain.
    """
    nt = tpc // 128
    npair = nt // 2
    nc = bacc.Bacc("TRN2", target_bir_lowering=False, debug=False)

    XH = nc.dram_tensor("XH", [D, tpc], F16, kind="ExternalInput")
    XL = nc.dram_tensor("XL", [D, tpc], F16, kind="ExternalInput")
    WH = nc.dram_tensor("WH", [D, E], F16, kind="ExternalInput")
    WL = nc.dram_tensor("WL", [D, E], F16, kind="ExternalInput")
    BIASR = nc.dram_tensor("BIASR", [128, E], F32, kind="ExternalInput")
    OW = nc.dram_tensor("OW", [tpc, TOPK], F32, kind="ExternalOutput")
    OI = nc.dram_tensor("OI", [tpc, TOPK], I32, kind="ExternalOutput")

    # DRAM views with the d-tile index split out: [128 p, n d-tiles, cols]
    xhv = XH.ap().rearrange("(n p) c -> p n c", p=128)
    xlv = XL.ap().rearrange("(n p) c -> p n c", p=128)
    whv = WH.ap().rearrange("(n p) c -> p n c", p=128)
    wlv = WL.ap().rearrange("(n p) c -> p n c", p=128)

    with tile.TileContext(nc) as tc:
        with (
            tc.tile_pool(name="consts", bufs=1) as consts,
            tc.tile_pool(name="wp", bufs=1) as wp,
            tc.tile_pool(name="xhp", bufs=3) as xhp,
            tc.tile_pool(name="xlp", bufs=2) as xlp,
            tc.tile_pool(name="rt", bufs=2) as rt,
            tc.tile_pool(name="psh", bufs=4, space="PSUM") as psh,
            tc.tile_pool(name="psl", bufs=2, space="PSUM") as psl,
            tc.tile_pool(name="psw", bufs=1, space="PSUM") as psw,
        ):
            biasr = consts.tile([128, E], F32)
            nc.sync.dma_start(biasr[:], BIASR[:])

            # PE p-state warm-up: ~11us of throwaway fp32 matmuls on biasr so
            # the HAM clock gate opens before the first real matmul arrives
            # (cold-start otherwise costs ~12us of 2-4x slower matmuls).
            scratch = psw.tile([128, E], F32)
            for _ in range(24):
                nc.tensor.matmul(scratch[:], biasr[:, 0:128], biasr[:, 0:E],
                                 start=True, stop=True)
            for _ in range(6):
                nc.tensor.matmul(scratch[:, 0:64], biasr[:, 0:128],
                                 biasr[:, 0:64], start=True, stop=True)

            wh = wp.tile([128, ND, E], F16)
            wl = wp.tile([128, ND, E], F16)
            H = ND // 2

            def load_w(half, lo):
                dst, src = (wl, wlv) if lo else (wh, whv)
                nc.sync.dma_start(
                    dst[:, half * H : (half + 1) * H, :],
                    src[:, half * H : (half + 1) * H, :],
                )

            xh_t = {}
            xl_t = {}

            def load_xh(j, half=None):
                if j not in xh_t:
                    xh_t[j] = xhp.tile([128, ND, 256], F16, tag="xh", name="xh")
                d0, d1 = (0, ND) if half is None else (half * H, (half + 1) * H)
                nc.sync.dma_start(
                    xh_t[j][:, d0:d1, :], xhv[:, d0:d1, bass.ts(j, 256)]
                )

            def load_xl(j):
                xl_t[j] = xlp.tile([128, ND, 256], F16, tag="xl", name="xl")
                nc.sync.dma_start(xl_t[j][:], xlv[:, :, bass.ts(j, 256)])

            hi_ps = {}
            lo_ps = {}

            def emit_hi(i, d0, d1):
                # per d-tile: xh.wh then xh.wl, same bank + accumulation
                # order as the proven baseline
                if i not in hi_ps:
                    hi_ps[i] = psh.tile([128, E], F32, tag="logits", name="logits")
                logits = hi_ps[i]
                xt = xh_t[i // 2]
                c = (i % 2) * 128
                for d in range(d0, d1):
                    nc.tensor.matmul(
                        logits[:], xt[:, d, c : c + 128], wh[:, d, :],
                        start=(d == 0), stop=False,
                    )
                    nc.tensor.matmul(
                        logits[:], xt[:, d, c : c + 128], wl[:, d, :],
                        start=False, stop=(d == ND - 1),
                    )

            def emit_lo(i):
                if i not in lo_ps:
                    lo_ps[i] = psl.tile([128, E], F32, tag="logits_lo",
                                        name="logits_lo")
                logits_lo = lo_ps[i]
                xt = xl_t[i // 2]
                c = (i % 2) * 128
                for d in range(ND):
                    nc.tensor.matmul(
                        logits_lo[:], xt[:, d, c : c + 128], wh[:, d, :],
                        start=(d == 0), stop=(d == ND - 1),
                    )

            def emit_cr(i):
                lo_sb = rt.tile([128, E], F32, tag="lo_sb", name="lo_sb", bufs=1)
                nc.scalar.copy(lo_sb[:], lo_ps.pop(i)[:])
                logsum = rt.tile([128, E], F32, tag="logsum", name="logsum", bufs=1)
                nc.vector.tensor_tensor(
                    logsum[:], hi_ps.pop(i)[:], lo_sb[:], op=ALU.add
                )
                scores = rt.tile([128, E], F32, tag="scores", name="scores", bufs=1)
                nc.scalar.activation(scores[:], logsum[:], AF.Sigmoid,
                                     scale=DESCALE)
                for t in _routing_thunks(nc, rt, scores, biasr, OW, OI, i):
                    t()

            # ---- startup DMA: the hi pass reads wh AND wl (d-interleaved,
            # proven accumulation order), so both W halves stream first,
            # chunked so pair-0 matmuls start after the first three chunks
            load_w(0, lo=False)
            load_w(0, lo=True)
            load_xh(0, half=0)
            load_w(1, lo=False)
            load_w(1, lo=True)
            load_xh(0, half=1)
            load_xh(1)
            load_xl(0)
            # pair 0 d-half-interleaved so the PE runs continuously from the
            # first WH/XH chunk arrival
            emit_hi(0, 0, H)
            emit_hi(1, 0, H)
            emit_hi(0, H, ND)
            emit_hi(1, H, ND)
            emit_hi(2, 0, ND)
            emit_hi(3, 0, ND)
            # ---- steady state: iter j handles lo+routing of pair j and the
            # hi pass of pair j+2 (lag-2), prefetching one pair ahead
            for j in range(npair):
                if j + 2 < npair:
                    load_xh(j + 2)
                if j + 1 < npair:
                    load_xl(j + 1)
                emit_lo(2 * j)
                emit_cr(2 * j)
                emit_lo(2 * j + 1)
                emit_cr(2 * j + 1)
                if j + 2 < npair:
                    emit_hi(2 * (j + 2), 0, ND)
                    emit_hi(2 * (j + 2) + 1, 0, ND)

    nc.compile()
    return nc


def kernel(x: np.ndarray, weight: np.ndarray, bias: np.ndarray):
    x = np.ascontiguousarray(x, dtype=np.float32)
    weight = np.ascontiguousarray(weight, dtype=np.float32)
    bias = np.ascontiguousarray(bias, dtype=np.float32)

    if "nc" not in _CACHE:
        _CACHE["nc"] = _build()
    nc = _CACHE["nc"]

    # host staging: transpose + fp16 hi/lo split (same RNE rounding the
    # device ACT/DVE casts produced in the previous revision)
    xs = x * np.float32(XSCALE)
    xh = xs.astype(np.float16)
    xl = (xs - xh.astype(np.float32)).astype(np.float16)
    ws = weight * np.float32(WSCALE)
    wh = ws.astype(np.float16)
    wl = (ws - wh.astype(np.float32)).astype(np.float16)

    wh_t = np.ascontiguousarray(wh.T)
    wl_t = np.ascontiguousarray(wl.T)
    biasr = np.tile(bias[None, :], (128, 1))

    in_maps = []
    for c in range(N_CORES):
        sl = slice(c * TPC, (c + 1) * TPC)
        in_maps.append(
            {
                "XH": np.ascontiguousarray(xh[sl].T),
                "XL": np.ascontiguousarray(xl[sl].T),
                "WH": wh_t,
                "WL": wl_t,
                "BIASR": biasr,
            }
        )
    global _last_in_maps
    _last_in_maps = in_maps
    res = run_bass_kernel_spmd(nc, in_maps, core_ids=list(range(N_CORES)))
    w = np.concatenate([r["OW"] for r in res.results], axis=0)
    idx = np.concatenate([r["OI"] for r in res.results], axis=0)
    return w, idx


_last_in_maps = None
